# revision 1
# baseline (speedup 1.0000x reference)
"""Trainium2 Bass kernel for nn_AiriaSNN: 3-layer LIF spiking net, 25 steps.

v3 design (pure data parallel over 8 cores, batch-sharded):
  - Per core Bc=131072 rows, F=512 cols/tile, 8 groups/tile -> 32 tiles,
    processed as 8 quads (4 tiles), two quads interleaved for pipelining.
  - L1 exact {0,1} arithmetic (matches reference rounding bit-for-bit):
      u  = beta*m1 + cur1          (DVE stt, [128, 2048])
      m1 = u - s1_prev             (GPSIMD stt, [128, 2048])
      s1 = (m1 > 1) -> bf16        (DVE tensor_scalar, 2x mode)
  - L2/L3 sigma-form {-1,0,+1} spikes with bias/reset constants folded into
    per-step thresholds; membrane tracked as n~ = m + D_t:
      p2 = W2hi@s1 + W2lo@s1 - sigma2_prev/2     (PE, fp16-hi + bf16-lo chunks)
      n2 = beta*n2 + p2                           (DVE stt from PSUM)
      sigma2 = Sign(n2 - th2_t)                   (ACT, per-partition bias)
    and same for L3 with W3/2 chunks.
  - Output sigma3 in bf16 {-1,0,+1}; host maps (y > 0) -> f32 {0,1}.
"""

import os
import sys

import numpy as np

_REPO = "/opt/trn_rl_repo"
if _REPO not in sys.path:
    sys.path.insert(0, _REPO)

import bass_rust
import concourse.bass as bass
import concourse.mybir as mybir
import concourse.tile as tile
from concourse.bass_utils import run_bass_kernel_spmd

BETA = 0.95
F32 = mybir.dt.float32
BF16 = mybir.dt.bfloat16
FP16 = mybir.dt.float16

N_CORES = 8
GROUPS = 8
F = 512


# ---------------------------------------------------------------------------
# Workaround: this walrus build rejects >1 sync waits on one instruction.
def _patched_drain_and_barrier(self, tick_clock, wait_clock):
    drain_inst = self.nc.sync.drain()
    wait_clock.add_sem_waits(
        drain_inst.ins, tile.ScopedClock({None: tick_clock.global_clock})
    )
    si = drain_inst.ins.sync_info
    if si is not None and len(si.on_wait) > 1:
        waits = list(si.on_wait)
        drain_inst.ins.sync_info = bass_rust.SyncInfo(
            on_wait=[], on_update=list(si.on_update)
        )
        for w in waits:
            nop = self.nc.sync.nop()
            nop.ins.sync_info = bass_rust.SyncInfo(on_wait=[w], on_update=[])
    self.nc.all_engine_barrier()
    assert self.sems is not None
    popped = self.nc._tile_sem_poison_stack.pop()
    assert popped is self._sem_poison
    self.nc.clear_and_free_semaphores(list(self.sems.allocated().values()))
    self.nc.all_engine_barrier()


tile.TileContext._drain_and_barrier = _patched_drain_and_barrier


def _split_excess_waits(nc, max_waits=1):
    for fn in nc.m.functions:
        for bb in fn.blocks:
            insts = bb.instructions
            pending = []
            for idx, inst in enumerate(insts):
                si = inst.sync_info
                if si is None or len(si.on_wait) <= max_waits:
                    continue
                waits = list(si.on_wait)
                keep = waits[-max_waits:]
                extra = waits[:-max_waits]
                nops = []
                for j in range(0, len(extra), max_waits):
                    nops.append(
                        mybir.InstNoOp(
                            name=nc.get_next_instruction_name(),
                            sync_info=mybir.SyncInfo(
                                on_wait=extra[j : j + max_waits], on_update=[]
                            ),
                            bass_nofuse=True,
                            engine=inst.engine,
                        )
                    )
                inst.sync_info = mybir.SyncInfo(
                    on_wait=keep, on_update=list(si.on_update)
                )
                pending.append((idx, nops))
            for idx, nops in reversed(pending):
                for nop in reversed(nops):
                    insts.insert(idx, nop)


# ---------------------------------------------------------------------------


def _blockdiag(w, groups):
    k, m = w.shape
    out = np.zeros((k * groups, m * groups), dtype=w.dtype)
    for g in range(groups):
        out[g * k : (g + 1) * k, g * m : (g + 1) * m] = w
    return out


class _Built:
    def __init__(self, nc, n_quads, T):
        self.nc = nc
        self.n_quads = n_quads
        self.T = T


_CACHE = {}

# Engine knobs (tuned on hardware)
L1_RESET_ENGINE = "gpsimd"  # "gpsimd" | "vector"


def build_nc(T, n_quads):
    key = (T, n_quads)
    if key in _CACHE:
        return _CACHE[key]

    nc = bass.Bass()
    Alu = mybir.AluOpType
    Act = mybir.ActivationFunctionType
    BETAf = float(np.float32(BETA))
    QF = 4 * F  # 2048: quad free size

    x_d = nc.dram_tensor("xprep", [4 * n_quads, 6 * GROUPS, F], F32, kind="ExternalInput")
    w1_d = nc.dram_tensor("w1t", [6 * GROUPS, 128], F32, kind="ExternalInput")
    b1_d = nc.dram_tensor("b1c", [128, 1], F32, kind="ExternalInput")
    w2hi_d = nc.dram_tensor("w2hi", [128, 64], FP16, kind="ExternalInput")
    w2lo_d = nc.dram_tensor("w2lo", [128, 64], BF16, kind="ExternalInput")
    w3hi_d = nc.dram_tensor("w3hi", [128, 48], FP16, kind="ExternalInput")
    w3lo_d = nc.dram_tensor("w3lo", [128, 48], BF16, kind="ExternalInput")
    neg2_d = nc.dram_tensor("neg2", [128, 128], BF16, kind="ExternalInput")
    th2_d = nc.dram_tensor("th2n", [128, T], F32, kind="ExternalInput")
    th3_d = nc.dram_tensor("th3n", [128, T], F32, kind="ExternalInput")
    y_d = nc.dram_tensor("y", [n_quads, T, 2, 48, F], BF16, kind="ExternalOutput")

    with tile.TileContext(nc) as tc:
        with (
            tc.tile_pool(name="const", bufs=1) as cpool,
            tc.tile_pool(name="state", bufs=2) as spool,
            tc.tile_pool(name="spk", bufs=3) as kpool,
            tc.tile_pool(name="io", bufs=3) as iopool,
            tc.tile_pool(name="psA", bufs=1, space="PSUM") as ppoolA,
            tc.tile_pool(name="psX", bufs=2, space="PSUM") as ppoolX,
        ):
            w1 = cpool.tile([6 * GROUPS, 128], F32)
            b1 = cpool.tile([128, 1], F32)
            w2hi = cpool.tile([128, 64], FP16)
            w2lo = cpool.tile([128, 64], BF16)
            w3hi = cpool.tile([128, 48], FP16)
            w3lo = cpool.tile([128, 48], BF16)
            neg2 = cpool.tile([128, 128], BF16)
            th2 = cpool.tile([128, T], F32)
            th3 = cpool.tile([128, T], F32)
            for t_, d_ in [
                (w1, w1_d), (b1, b1_d), (w2hi, w2hi_d), (w2lo, w2lo_d),
                (w3hi, w3hi_d), (w3lo, w3lo_d), (neg2, neg2_d),
                (th2, th2_d), (th3, th3_d),
            ]:
                nc.sync.dma_start(t_[:], d_[:])

            def reset_engine():
                return nc.gpsimd if L1_RESET_ENGINE == "gpsimd" else nc.vector

            for qq in range(0, n_quads, 2):
                quads = [qq, qq + 1]
                st = {}
                for qi, q in enumerate(quads):
                    cur1 = spool.tile([128, QF], F32, tag=f"cur1_{qi}")
                    m1 = spool.tile([128, QF], F32, tag=f"m1_{qi}")
                    n2 = spool.tile([128, 2 * F], F32, tag=f"n2_{qi}")
                    n3 = spool.tile([128, F], F32, tag=f"n3_{qi}")
                    # x load + cur1 = W1 @ x + b1 (exact fp32 matmul)
                    for j in range(4):
                        ti = 4 * q + j
                        x_t = iopool.tile([6 * GROUPS, F], F32, tag=f"x_{qi}")
                        nc.sync.dma_start(x_t[:], x_d[ti][:])
                        pc1 = ppoolX.tile([128, F], F32, tag="pc1")
                        nc.tensor.matmul(pc1[:], w1[:], x_t[:], start=True, stop=True)
                        nc.scalar.activation(
                            cur1[:, j * F : (j + 1) * F], pc1[:],
                            Act.Identity, bias=b1[:, 0:1],
                        )
                    st[qi] = dict(cur1=cur1, m1=m1, n2=n2, n3=n3,
                                  s1p=None, sg2p=None, sg3p=None)

                for t in range(T):
                    for qi, q in enumerate(quads):
                        S = st[qi]
                        cur1, m1, n2, n3 = S["cur1"], S["m1"], S["n2"], S["n3"]

                        # ---- L1 (exact) ----
                        if t == 0:
                            mm1 = cur1  # m1_0 == cur1
                        else:
                            src = cur1 if t == 1 else m1
                            nc.vector.scalar_tensor_tensor(
                                m1[:], src[:], BETAf, cur1[:], Alu.mult, Alu.add
                            )
                            reset_engine().tensor_tensor(
                                m1[:], m1[:], S["s1p"][:], Alu.subtract
                            )
                            mm1 = m1
                        s1 = kpool.tile([128, QF], BF16, tag=f"s1_{qi}")
                        nc.vector.tensor_scalar(
                            s1[:], mm1[:], 1.0, None, Alu.is_gt
                        )

                        # ---- L2: p2 = W2@s1 - sigma2p/2 ----
                        p2 = ppoolA.tile([128, 2 * F], F32, tag=f"p2_{qi}")
                        for p in range(2):
                            for h in range(2):
                                rhs = s1[:, (2 * p + h) * F : (2 * p + h + 1) * F]
                                out = p2[64 * h : 64 * h + 64, p * F : (p + 1) * F]
                                nc.tensor.matmul(
                                    out, w2hi[:], rhs, start=True, stop=False,
                                    tile_position=(0, 64 * h),
                                )
                                nc.tensor.matmul(
                                    out, w2lo[:], rhs, start=False,
                                    stop=(t == 0),
                                    tile_position=(0, 64 * h),
                                )
                        if t > 0:
                            for p in range(2):
                                nc.tensor.matmul(
                                    p2[:, p * F : (p + 1) * F], neg2[:],
                                    S["sg2p"][:, p * F : (p + 1) * F],
                                    start=False, stop=True, skip_group_check=True,
                                )
                        if t == 0:
                            nc.scalar.activation(n2[:], p2[:], Act.Copy)
                        else:
                            nc.vector.scalar_tensor_tensor(
                                n2[:], n2[:], BETAf, p2[:], Alu.mult, Alu.add
                            )
                        sg2 = kpool.tile([128, 2 * F], BF16, tag=f"sg2_{qi}")
                        nc.scalar.activation(
                            sg2[:], n2[:], Act.Sign, bias=th2[:, t : t + 1]
                        )

                        # ---- L3: p3 = (W3/2)@sigma2 - sigma3p/2 ----
                        p3 = ppoolA.tile([128, F], F32, tag=f"p3_{qi}")
                        for p in range(2):
                            rhs = sg2[:, p * F : (p + 1) * F]
                            out = p3[64 * p : 64 * p + 48, :]
                            nc.tensor.matmul(
                                out, w3hi[:], rhs, start=True, stop=False,
                                tile_position=(0, 64 * p),
                            )
                            nc.tensor.matmul(
                                out, w3lo[:], rhs, start=False,
                                stop=(t == 0),
                                tile_position=(0, 64 * p),
                            )
                        if t > 0:
                            nc.tensor.matmul(
                                p3[:], neg2[:], S["sg3p"][:],
                                start=False, stop=True, skip_group_check=True,
                            )
                        if t == 0:
                            nc.scalar.activation(n3[:], p3[:], Act.Copy)
                        else:
                            nc.vector.scalar_tensor_tensor(
                                n3[:], n3[:], BETAf, p3[:], Alu.mult, Alu.add
                            )
                        sg3 = kpool.tile([128, F], BF16, tag=f"sg3_{qi}")
                        nc.scalar.activation(
                            sg3[:], n3[:], Act.Sign, bias=th3[:, t : t + 1]
                        )
                        for p in range(2):
                            nc.sync.dma_start(
                                y_d[q][t][p][:], sg3[64 * p : 64 * p + 48, :]
                            )
                        S["s1p"] = s1
                        S["sg2p"] = sg2
                        S["sg3p"] = sg3

    _split_excess_waits(nc)
    built = _Built(nc, n_quads, T)
    _CACHE[key] = built
    return built


def _prep_consts(W1, b1, W2, b2, W3, b3, T):
    import ml_dtypes

    w1t = _blockdiag(np.ascontiguousarray(W1.T), GROUPS)  # [48, 128]
    b1c = np.tile(b1, GROUPS).reshape(-1, 1).astype(np.float32)

    # W2 chunks: hi fp16, lo bf16 of blockdiag(W2.T x 8)
    w2t = _blockdiag(np.ascontiguousarray(W2.T), GROUPS)  # [128, 64]
    w2hi = w2t.astype(np.float16)
    w2lo = (w2t - w2hi.astype(np.float32)).astype(ml_dtypes.bfloat16)
    # W3/2 chunks: blockdiag(W3.T/2 x 16) [128, 48]
    w3t = _blockdiag(np.ascontiguousarray(W3.T) * 0.5, 16)
    w3hi = w3t.astype(np.float16)
    w3lo = (w3t - w3hi.astype(np.float32)).astype(ml_dtypes.bfloat16)

    neg2 = (-0.5 * np.eye(128)).astype(ml_dtypes.bfloat16)

    # Threshold schedules: th[:, t] = -(1 + D_t), ACT computes Sign(n + th),
    # where n~ = m + D tracks the membrane with bias/reset consts folded out:
    # D2_0 = -b2; D2_t = beta*D2 - b2 + 1/2
    # D3_0 = -(b3 + w3s); D3_t = beta*D3 - (b3 + w3s) + 1/2  (w3s = rowsum W3/2)
    b = np.float64(np.float32(BETA))
    th2 = np.zeros((128, T), np.float32)
    th3 = np.zeros((128, T), np.float32)
    w3s = (W3.astype(np.float64) * 0.5).sum(axis=1)  # [3]
    D2 = -b2.astype(np.float64)
    D3 = -(b3.astype(np.float64) + w3s)
    p128 = np.arange(128)
    r64 = p128 % 64
    for t in range(T):
        if t > 0:
            D2 = b * D2 - b2.astype(np.float64) + 0.5
            D3 = b * D3 - (b3.astype(np.float64) + w3s) + 0.5
        th2[:, t] = (-(1.0 + D2[p128 % 8])).astype(np.float32)
        # L3 rows within each 64 block: r = 24h + 3g + n3 for r < 48
        th3[:, t] = np.where(
            r64 < 48, (-(1.0 + D3[r64 % 3])).astype(np.float32), np.float32(0)
        )
    return dict(w1t=w1t, b1c=b1c, w2hi=w2hi, w2lo=w2lo, w3hi=w3hi, w3lo=w3lo,
                neg2=neg2, th2n=th2, th3n=th3)


def kernel(x, W1, b1, W2, b2, W3, b3, num_steps):
    return _run(x, W1, b1, W2, b2, W3, b3, num_steps, trace=False)[0]


def kernel_profiled(x, W1, b1, W2, b2, W3, b3, num_steps):
    return _run(x, W1, b1, W2, b2, W3, b3, num_steps, trace=True)


def _run(x, W1, b1, W2, b2, W3, b3, num_steps, trace=False):
    x = np.asarray(x)
    T = int(num_steps)
    B = x.shape[0]
    assert B % N_CORES == 0
    Bc = B // N_CORES
    assert Bc % (GROUPS * F) == 0
    n_tiles = Bc // (GROUPS * F)
    assert n_tiles % 4 == 0
    n_quads = n_tiles // 4

    built = build_nc(T, n_quads)
    consts = _prep_consts(
        np.asarray(W1), np.asarray(b1), np.asarray(W2), np.asarray(b2),
        np.asarray(W3), np.asarray(b3), T,
    )

    in_maps = []
    for c in range(N_CORES):
        xc = x[c * Bc : (c + 1) * Bc]
        xp = np.ascontiguousarray(
            xc.reshape(n_tiles, GROUPS, F, 6).transpose(0, 1, 3, 2)
        ).reshape(n_tiles, 6 * GROUPS, F)
        m = {"xprep": xp}
        m.update(consts)
        in_maps.append(m)

    res = run_bass_kernel_spmd(built.nc, in_maps, list(range(N_CORES)), trace=trace)

    out = np.empty((T, B, 3), dtype=np.float32)
    for c in range(N_CORES):
        y = res.results[c]["y"]  # [n_quads, T, 2, 48, F] bf16 {-1,0,1}
        s = (np.asarray(y) > 0)
        # rows: 24h + 3g + n3 -> reshape [nq, T, 2, 2, 8, 3, F]
        s = s.reshape(n_quads, T, 2, 2, 8, 3, F)
        # batch = ((q*4 + 2p + h)*8 + g)*F + f ; want [T, q, p, h, g, f, n]
        s = s.transpose(1, 0, 2, 3, 4, 6, 5).reshape(T, Bc, 3)
        out[:, c * Bc : (c + 1) * Bc, :] = s.astype(np.float32)
    return out, res

